# revision 23
# baseline (speedup 1.0000x reference)
"""Trainium2 Bass kernel for per-position head-mixing attention.

Math (per position p): Qh,Kh,Vh = reshape(q/k/v[p], [16, 64]);
L = Qh @ Kh.T / 8; W = softmax(L, axis=-1); out[p] = W @ Vh.

Strategy:
  * Pure data parallel over 8 cores (4096 positions each).
  * Host pre-transposes q,k to [d, ...] layout (exact, host-side) and casts to
    bf16 so every device DMA is large and contiguous.
  * Device: per group of 8 positions, one 73x128x128 matmul computes all
    16x16 logit blocks; 9 extra contraction rows add -C to off-diagonal
    (cross-position) entries so exp() zeroes them - no masking op needed.
    The 9 mask rows live at partitions 64..72 of a persistent double buffer,
    written once; per-tile DMAs refresh only partitions 0..63.
  * exp on ScalarE (scale=1/8 fused), batched over 8 groups.
  * Second matmul W' @ [V | 1] gives numerator and softmax denominator in one
    pass; VectorE reciprocal+multiply normalizes (fp16 out).
  * DMA routing: big loads on gpsimd/SWDGE (16-engine spray), output store on
    the sync HWDGE ring so stores never block loads.
"""

import sys

if "/opt/trn_rl_repo" not in sys.path:
    sys.path.insert(0, "/opt/trn_rl_repo")

from contextlib import ExitStack

import ml_dtypes
import numpy as np

import concourse.bass as bass
from concourse import bacc, mybir, tile
from concourse.bass_utils import run_bass_kernel_spmd

BF16 = mybir.dt.bfloat16
F16 = mybir.dt.float16
F32 = mybir.dt.float32
NPBF16 = ml_dtypes.bfloat16

N_CORES = 8
S_TOT = 4 * 8192          # flattened (batch, seq) positions
H, D = 16, 64             # heads, key size
N_PC = S_TOT // N_CORES   # positions per core
T = 512                   # positions per on-chip tile
G = T // 8                # 8-position groups per tile
G2 = G // 2               # groups per SBUF partition-half buffer
NT = N_PC // T            # tiles per core
B = 8                     # groups per psum/exp batch
NB = G // B
C_MASK = 400.0            # off-diagonal logit penalty (exact in bf16)
SCALE = 0.125             # 1/sqrt(64)
QCH = 4160                # qk DRAM chunk cols (4096 data + pad -> 8320 B,
                          # matching the v layout whose DMAs run at line rate)

_CACHE = {}


def _build_program(nt: int, n_cores: int):
    nc = bacc.Bacc(
        "TRN2", target_bir_lowering=False, debug=False, num_devices=n_cores
    )
    # combined q|k data, chunk-major: one [128, QCH] chunk per (tile, side),
    # rows 0-63 = d-rows of groups 0..G2-1, rows 64-127 = groups G2..; the
    # 16*8320 B row pitch matches the v layout, whose DMAs run at line rate
    # (power-of-2-ish partition strides measurably halve per-engine DMA rate)
    qk = nc.dram_tensor("qk", [128, nt * 2 * QCH], BF16, kind="ExternalInput").ap()
    mk = nc.dram_tensor("mk", [9, 2, G2, H, 8], BF16, kind="ExternalInput").ap()
    vr = nc.dram_tensor("vr", [H, 8, nt, G, 65], BF16, kind="ExternalInput").ap()
    out = nc.dram_tensor("o", [H, 8, nt, G, D], F16, kind="ExternalOutput").ap()

    HW2 = G2 * 128  # columns per side per half-tile

    with tile.TileContext(nc) as tc, ExitStack() as ctx:
        qk_pool = ctx.enter_context(tc.tile_pool(name="qk", bufs=1))
        v_pool = ctx.enter_context(tc.tile_pool(name="v", bufs=3))
        o_pool = ctx.enter_context(tc.tile_pool(name="o", bufs=3))
        w_pool = ctx.enter_context(tc.tile_pool(name="w", bufs=3))
        r_pool = ctx.enter_context(tc.tile_pool(name="r", bufs=3))
        p1_pool = ctx.enter_context(tc.tile_pool(name="p1", bufs=2, space="PSUM"))
        p2_pool = ctx.enter_context(tc.tile_pool(name="p2", bufs=2, space="PSUM"))

        # The DMA port swizzle maps SBUF partitions 0-63 to the 8 even ports
        # and 64-127 to the odd ones, so any 64-partition stream caps at half
        # bandwidth. Each tile is therefore split across both halves: groups
        # 0..G2-1 live in an A buffer (data rows 0-63, mask rows 64-72, K=73
        # matmuls), groups G2.. in a B buffer (data rows 64-127, mask rows
        # 0-8, zeros 9-63, K=128 matmuls - zero rows contribute nothing).
        # A/B DMAs are interleaved so both port groups run concurrently.
        NB_QK = 3  # qk pipeline depth (tiles in flight)
        bufsA = [
            qk_pool.tile([73, 2 * HW2], BF16, tag=f"qkA{p}", name=f"qkA{p}")
            for p in range(NB_QK)
        ]
        bufsB = [
            qk_pool.tile([128, 2 * HW2], BF16, tag=f"qkB{p}", name=f"qkB{p}")
            for p in range(NB_QK)
        ]
        for p in range(NB_QK):
            nc.vector.memset(bufsB[p][0:64, :], 0.0)
        mflat = mk.rearrange("p a b c d -> p (a b c d)")  # [9, 2*HW2]

        def load_mask(buf, rows):
            for c in range(2):
                lo, hi = c * HW2, (c + 1) * HW2
                nc.gpsimd.dma_start(buf[rows[0] : rows[1], lo:hi], mflat[:, lo:hi])

        def load_qk(i):
            A, Bf = bufsA[i % NB_QK], bufsB[i % NB_QK]
            for side in range(2):
                c0 = (i * 2 + side) * QCH
                nc.scalar.dma_start(
                    A[0:64, side * HW2 : (side + 1) * HW2],
                    qk[0:64, c0 : c0 + HW2],
                )
                nc.scalar.dma_start(
                    Bf[64:128, side * HW2 : (side + 1) * HW2],
                    qk[64:128, c0 : c0 + HW2],
                )

        def load_v(i):
            v_t = v_pool.tile([128, G * 65], BF16)
            nc.gpsimd.dma_start(v_t[:], vr[:, :, i].rearrange("k p g e -> (k p) (g e)"))
            return v_t

        load_mask(bufsA[0], (64, 73))
        load_qk(0)
        load_mask(bufsB[0], (0, 9))
        for p in range(1, NB_QK):
            load_mask(bufsA[p], (64, 73))
            load_mask(bufsB[p], (0, 9))
        v_tiles = {0: load_v(0)}
        load_qk(1)
        v_tiles[1] = load_v(1)

        for i in range(nt):
            # prefetch next tile's inputs ahead of this tile's compute so the
            # gpsimd DMA queue never sits behind compute-dependent work
            if i + 2 < nt:
                load_qk(i + 2)
                v_tiles[i + 2] = load_v(i + 2)
            tA, tB = bufsA[i % NB_QK], bufsB[i % NB_QK]
            v_t = v_tiles.pop(i)
            o_t = o_pool.tile([128, G * 64], F16)
            oflat = out[:, :, i].rearrange("k p g e -> (k p) (g e)")

            stash = None
            for b in range(NB + 1):
                if b < NB:
                    p1 = p1_pool.tile([128, B * 128], F32)
                    w = w_pool.tile([128, B * 128], BF16)
                    # exp is issued per half-batch so it starts before all 8
                    # matmuls finish and is long done when mm2 needs it
                    hw_cols = B * 128 // 2
                    for half in range(2):
                        for j in range(half * (B // 2), (half + 1) * (B // 2)):
                            g = b * B + j
                            tb, goff = (tA, g) if g < G2 else (tB, g - G2)
                            nc.tensor.matmul(
                                p1[:, j * 128 : (j + 1) * 128],
                                lhsT=tb[
                                    :, HW2 + goff * 128 : HW2 + (goff + 1) * 128
                                ],
                                rhs=tb[:, goff * 128 : (goff + 1) * 128],
                                start=True,
                                stop=True,
                            )
                        nc.scalar.activation(
                            w[:, half * hw_cols : (half + 1) * hw_cols],
                            p1[:, half * hw_cols : (half + 1) * hw_cols],
                            mybir.ActivationFunctionType.Exp,
                            scale=SCALE,
                        )
                else:
                    w = None
                if stash is not None:
                    wp, bp = stash
                    p2 = p2_pool.tile([128, B * 128], F32)
                    for j in range(B):
                        g = bp * B + j
                        nc.tensor.matmul(
                            p2[:, j * 128 : j * 128 + 65],
                            lhsT=wp[:, j * 128 : (j + 1) * 128],
                            rhs=v_t[:, g * 65 : (g + 1) * 65],
                            start=True,
                            stop=True,
                        )
                    r = r_pool.tile([128, B], F32)
                    p2v = p2[:].rearrange("p (g c) -> p g c", c=128)
                    nc.vector.reciprocal(r[:], p2v[:, :, 64])
                    rb = r[:].unsqueeze(2).broadcast_to([128, B, 64])
                    ov = o_t[:, bp * B * 64 : (bp + 1) * B * 64].rearrange(
                        "p (g c) -> p g c", c=64
                    )
                    nc.vector.tensor_tensor(
                        ov, p2v[:, :, 0:64], rb, op=mybir.AluOpType.mult
                    )
                if stash is not None and bp == NB // 2 - 1:
                    nc.sync.dma_start(oflat[:, : G * 32], o_t[:, : G * 32])
                stash = (w, b) if w is not None else None
            nc.sync.dma_start(oflat[:, G * 32 :], o_t[:, G * 32 :])

    nc.compile()
    return nc


def _prep_qk(qslab: np.ndarray, kslab: np.ndarray, nt: int) -> np.ndarray:
    """Two [nt*T, 1024] fp32 slabs -> [128, nt*2*QCH] bf16 chunk-major."""
    full = np.zeros((128, nt, 2, QCH), dtype=NPBF16)
    dv = full[:, :, :, : G2 * 128].reshape(128, nt, 2, G2, H, 8)
    for s, slab in enumerate((qslab, kslab)):
        a = slab.reshape(nt, 2, G2, 8, H, D).astype(NPBF16)
        t = a.transpose(5, 0, 1, 2, 4, 3)           # [d, i, half, g', h, p]
        dv[0:64, :, s] = t[:, :, 0]
        dv[64:128, :, s] = t[:, :, 1]
    return full.reshape(128, nt * 2 * QCH)


def _mask_const() -> np.ndarray:
    """[9, 2, G2, H, 8] bf16: mask rows of a half-tile buffer."""
    m = np.zeros((9, 2, G2, H, 8), dtype=NPBF16)
    m[0, 0] = NPBF16(1.0)        # q side, row 64: ones
    m[0, 1] = NPBF16(-C_MASK)    # k side, row 64: -C
    for j in range(8):
        m[1 + j, 0, :, :, j] = NPBF16(1.0)     # q side: delta(p, j)
        m[1 + j, 1, :, :, j] = NPBF16(C_MASK)  # k side: C * delta(p, j)
    return m


def _prep_v(slab: np.ndarray, nt: int) -> np.ndarray:
    """[nt*T, 1024] fp32 -> [H, 8, nt, G, 65] bf16 with ones column."""
    a = slab.reshape(nt, G, 8, H, D)
    full = np.empty((H, 8, nt, G, 65), dtype=NPBF16)
    full[..., :64] = a.transpose(3, 2, 0, 1, 4).astype(NPBF16)
    full[..., 64] = NPBF16(1.0)
    return full


def kernel(q: np.ndarray, k: np.ndarray, v: np.ndarray) -> np.ndarray:
    bshape = q.shape
    qf = np.ascontiguousarray(np.asarray(q, dtype=np.float32)).reshape(S_TOT, H * D)
    kf = np.ascontiguousarray(np.asarray(k, dtype=np.float32)).reshape(S_TOT, H * D)
    vf = np.ascontiguousarray(np.asarray(v, dtype=np.float32)).reshape(S_TOT, H * D)

    key = (NT, N_CORES)
    if key not in _CACHE:
        _CACHE[key] = _build_program(*key)
    nc = _CACHE[key]

    mk = _mask_const()
    in_maps = []
    for c in range(N_CORES):
        s0, s1 = c * N_PC, (c + 1) * N_PC
        in_maps.append(
            {
                "qk": _prep_qk(qf[s0:s1], kf[s0:s1], NT),
                "mk": mk,
                "vr": _prep_v(vf[s0:s1], NT),
            }
        )

    res = run_bass_kernel_spmd(nc, in_maps, core_ids=list(range(N_CORES)))

    out = np.empty((S_TOT, H * D), dtype=np.float32)
    for c in range(N_CORES):
        o = res.results[c]["o"]  # [H, 8, NT, G, D] fp16
        out[c * N_PC : (c + 1) * N_PC] = (
            o.transpose(2, 3, 1, 0, 4).reshape(N_PC, H * D).astype(np.float32)
        )
    return out.reshape(bshape)

